# revision 1
# baseline (speedup 1.0000x reference)
"""Cross-temporal attention kernel for Trainium2 (8 NeuronCores, SPMD).

Problem (per batch b):
    q = Wq @ post + bq          (32, N)     N = 64*64 = 4096
    k = Wk @ pre  + bk          (32, N)
    v = Wv @ pre  + bv          (256, N)
    att = softmax_j(q^T k)      (N, N)
    out = gamma * (v @ att^T) + post

Sharding: 8 cores = 4 batches x 2 query-halves. Each core holds the full
pre[b] (for k/v) and its half of post[b] (for q + residual), and computes
a (256, 2048) slice of the output.

Device algorithm (per core). Everything that streams through the PE is
bf16 (1 column/cycle, fp32 PSUM accumulation, no PE mode switches),
software-pipelined: at step g the PE streams S^T(g) and AV(g-2) while
ScalarE runs exp(g-1).

Bias handling (exact): softmax_j is invariant to per-query additive
terms, so of the four terms of (k+bk)^T(q+bq) only the per-key term
t2[j] = sum_d bq[d]*k_raw[d,j] matters. t2 is produced as an extra
column of the vT projection ((Wk^T bq) appended to the v weights) and
applied as the per-partition bias of the exp activation. The q/k
projections are then bias-free, so their PSUM evacuations are plain
copies split between ScalarE and VectorE.

Denominator: fp32 adds split across VectorE/GpSimd for j-blocks 0..27;
the last four j-blocks are folded into the bf16 ones-matmul that also
broadcasts the total across partitions; reciprocal_approx_fast; final
out = avout*recip + gamma*bv + post, pipelined per 512-column half.
Each i-chunk's tail is deferred into the next chunk's pipeline head.

gamma and bv are folded into the v-projection weights on the host
(exact algebra: gamma*(v@att) = ((gamma*Wv)pre)@att + gamma*bv*denom).
"""

import numpy as np
import ml_dtypes

_CACHE = {}

B, C, HH, WW = 4, 256, 64, 64
N = HH * WW          # 4096 keys per batch
NI = N // 2          # 2048 queries per core
NCORES = 8
IC = 1024            # i-chunk (queries per inner tile)
NICHUNK = NI // IC   # 2
NJB = N // 128       # 32 j-blocks
NJB_ACC = NJB - 4    # j-blocks accumulated on DVE/GpSimd (rest on PE)
VW = 258             # vT tile width: 256 channels + t2 column + pad
BF16 = ml_dtypes.bfloat16
# denominator accumulation split (DVE ~2.2x faster than GpSimd per op);
# GpSimd gets early blocks plus alternating late ones so neither serial
# chain trails the loop end
GPS_JB = frozenset({3, 5, 6, 7, 11, 13, 14, 15, 19})
DVE_JB = frozenset(jb for jb in range(NJB_ACC) if jb not in GPS_JB)

# packed weight buffer layout (per-partition bytes, 128 partitions):
#   wqt 2*128 bf16 = 512B | wkt 512B | wvt 2*258 bf16 = 1032B |
#   gbv 2*4B | total 2064B
WPACK_BYTES = 2064


def _build_program():
    from contextlib import ExitStack
    from concourse import bacc, tile, mybir

    f32 = mybir.dt.float32
    bf16 = mybir.dt.bfloat16
    u8 = mybir.dt.uint8
    ADD = mybir.AluOpType.add
    EXP = mybir.ActivationFunctionType.Exp
    COPY = mybir.ActivationFunctionType.Copy

    nc = bacc.Bacc("TRN2", target_bir_lowering=False, debug=False,
                   num_devices=NCORES)

    pre_d = nc.dram_tensor("pre", [C, N], bf16, kind="ExternalInput").ap()
    postr_d = nc.dram_tensor("postr", [C, NI], bf16, kind="ExternalInput").ap()
    post_d = nc.dram_tensor("post", [C, NI], f32, kind="ExternalInput").ap()
    wpk_d = nc.dram_tensor("wpk", [128, WPACK_BYTES], u8,
                           kind="ExternalInput").ap()
    ones_d = nc.dram_tensor("ones", [128, 128], bf16, kind="ExternalInput").ap()
    out_d = nc.dram_tensor("out", [C, NI], f32, kind="ExternalOutput").ap()

    with tile.TileContext(nc) as tc:
        with ExitStack() as ctx:
            consts = ctx.enter_context(tc.tile_pool(name="consts", bufs=1))
            bigs = ctx.enter_context(tc.tile_pool(name="bigs", bufs=1))

            pre_s = bigs.tile([128, 2, N], bf16)
            postr_s = bigs.tile([128, 2, NI], bf16)
            post_s = bigs.tile([128, 2, NI], f32)
            wpk_s = consts.tile([128, WPACK_BYTES], u8)
            ones_s = consts.tile([128, 128], bf16)

            # ring 1 (Sync HW-DGE), ordered by first use
            nc.sync.dma_start(out=wpk_s, in_=wpk_d)
            for kc in range(2):
                nc.sync.dma_start(
                    out=pre_s[:, kc, 0:2048],
                    in_=pre_d[kc * 128:(kc + 1) * 128, 0:2048])
            for kc in range(2):
                nc.sync.dma_start(
                    out=postr_s[:, kc, 0:IC],
                    in_=postr_d[kc * 128:(kc + 1) * 128, 0:IC])
            for kc in range(2):
                nc.sync.dma_start(
                    out=pre_s[:, kc, 2048:N],
                    in_=pre_d[kc * 128:(kc + 1) * 128, 2048:N])
            for kc in range(2):
                nc.sync.dma_start(
                    out=postr_s[:, kc, IC:NI],
                    in_=postr_d[kc * 128:(kc + 1) * 128, IC:NI])
            # residual input is not needed until the first normalize
            # (~halfway through the kernel): keep it last on ring 1 so it
            # never competes with the critical pre/postr loads
            for kc in range(2):
                nc.sync.dma_start(out=post_s[:, kc, :],
                                  in_=post_d[kc * 128:(kc + 1) * 128, :])
            # ring 2 (Scalar HW-DGE): just the tiny ones matrix
            nc.scalar.dma_start(out=ones_s, in_=ones_d)

            # views into the packed weight buffer
            wqt_s = wpk_s[:, 0:512].bitcast(bf16).rearrange(
                "p (kc m) -> p kc m", kc=2)
            wkt_s = wpk_s[:, 512:1024].bitcast(bf16).rearrange(
                "p (kc m) -> p kc m", kc=2)
            wvt_s = wpk_s[:, 1024:2056].bitcast(bf16).rearrange(
                "p (kc m) -> p kc m", kc=2)
            gbv_s = wpk_s[:, 2056:2064].bitcast(f32)

            q_s = bigs.tile([128, NI], bf16)
            k_s = bigs.tile([128, N], bf16)
            vt_s = bigs.tile([128, NJB, VW], bf16)

            # ---- projections (emitted in data-arrival order; PSUM
            # evacuations alternate between ScalarE and VectorE) ----
            with tc.tile_pool(name="proj_psum", bufs=4, space="PSUM") as pp:
                def evac(idx, dst, src):
                    if idx % 2 == 0:
                        nc.vector.tensor_copy(dst, src)
                    else:
                        nc.scalar.activation(dst, src, COPY)

                def kproj(t):
                    ps = pp.tile([128, 512], f32, tag="ps", name=f"psk{t}")
                    for kc in range(2):
                        nc.tensor.matmul(
                            ps,
                            lhsT=wkt_s[:, kc, :],
                            rhs=pre_s[:, kc, t * 512:(t + 1) * 512],
                            start=(kc == 0), stop=(kc == 1))
                    evac(t, k_s[:, t * 512:(t + 1) * 512], ps)

                def qproj(t):
                    ps = pp.tile([128, 512], f32, tag="ps", name=f"psq{t}")
                    for kc in range(2):
                        nc.tensor.matmul(
                            ps,
                            lhsT=wqt_s[:, kc, :],
                            rhs=postr_s[:, kc, t * 512:(t + 1) * 512],
                            start=(kc == 0), stop=(kc == 1))
                    evac(t, q_s[:, t * 512:(t + 1) * 512], ps)

                def vproj(jb):
                    ps = pp.tile([128, 512], f32, tag="ps", name=f"psv{jb}")
                    for kc in range(2):
                        nc.tensor.matmul(
                            ps[:, 0:VW],
                            lhsT=pre_s[:, kc, jb * 128:(jb + 1) * 128],
                            rhs=wvt_s[:, kc, :],
                            start=(kc == 0), stop=(kc == 1))
                    evac(jb, vt_s[:, jb, :], ps[:, 0:VW])

                for t in range(4):          # k front (pre 0:2048)
                    kproj(t)
                for t in range(2):          # q front (postr 0:1024)
                    qproj(t)
                for jb in range(16):        # vT front
                    vproj(jb)
                for t in range(4, 8):       # k back
                    kproj(t)
                for jb in range(16, 32):    # vT back
                    vproj(jb)
                for t in range(2, 4):       # q back (postr back arrives last)
                    qproj(t)

            # ---- attention (two-stage pipeline over j-blocks) ----
            ppool = ctx.enter_context(tc.tile_pool(name="pchunk", bufs=11))
            dpool = ctx.enter_context(tc.tile_pool(name="dacc", bufs=2))
            rpool = ctx.enter_context(tc.tile_pool(name="recipb", bufs=2))
            opool = ctx.enter_context(tc.tile_pool(name="outsb", bufs=4))
            s_psum = ctx.enter_context(
                tc.tile_pool(name="s_psum", bufs=2, space="PSUM"))
            o_psum = ctx.enter_context(
                tc.tile_pool(name="o_psum", bufs=2, space="PSUM"))

            pending = None  # previous chunk's deferred tail state

            def emit_denom_head(st):
                # bf16 ones-matmul: broadcasts the denominator across
                # partitions, folding in the tail j-blocks' p. The head
                # (all but the last p) carries no dependency on the final
                # exp, so the PE can run it while exp(31) is in flight.
                dps = s_psum.tile([128, IC], f32, tag="sp",
                                  name=f"dps{st['ic']}")
                st['dps'] = dps
                for h in range(2):
                    hs = slice(h * 512, (h + 1) * 512)
                    nc.tensor.matmul(dps[:, hs], lhsT=ones_s,
                                     rhs=st['daccr'][:, hs],
                                     start=True, stop=False,
                                     skip_group_check=True)
                    for pt in st['ptail'][:-1]:
                        nc.tensor.matmul(dps[:, hs], lhsT=ones_s,
                                         rhs=pt[:, hs],
                                         start=False, stop=False,
                                         skip_group_check=True)

            def emit_denom_tail(st):
                for h in range(2):
                    hs = slice(h * 512, (h + 1) * 512)
                    nc.tensor.matmul(st['dps'][:, hs], lhsT=ones_s,
                                     rhs=st['ptail'][-1][:, hs],
                                     start=False, stop=True,
                                     skip_group_check=True)

            def emit_denom(st):
                emit_denom_head(st)
                emit_denom_tail(st)

            def emit_recips(st):
                st['rb'] = []
                for h in range(2):
                    hs = slice(h * 512, (h + 1) * 512)
                    rb = rpool.tile([128, 512], f32, tag="rb",
                                    name=f"rb{st['ic']}_{h}")
                    nc.vector.reciprocal_approx_fast(out=rb,
                                                     in_=st['dps'][:, hs])
                    st['rb'].append(rb)

            def emit_normalize(st, h):
                ic, i0 = st['ic'], st['ic'] * IC
                hs = slice(h * 512, (h + 1) * 512)
                rb = st['rb'][h]
                for cb in range(2):
                    osb = opool.tile([128, 512], f32, tag="osb",
                                     name=f"osb{ic}_{cb}_{h}")
                    nc.vector.tensor_mul(osb, st['ops'][cb][:, hs], rb)
                    nc.vector.scalar_tensor_tensor(
                        out=osb, in0=osb, scalar=gbv_s[:, cb:cb + 1],
                        in1=post_s[:, cb, i0 + h * 512:i0 + (h + 1) * 512],
                        op0=ADD, op1=ADD)
                    nc.sync.dma_start(
                        out=out_d[cb * 128:(cb + 1) * 128,
                                  i0 + h * 512:i0 + (h + 1) * 512],
                        in_=osb)

            for ic in range(NICHUNK):
                i0 = ic * IC
                dacc_v = dpool.tile([128, IC], f32, tag="dacc_v",
                                    name=f"dacc_v{ic}")
                dacc_g = dpool.tile([128, IC], f32, tag="dacc_g",
                                    name=f"dacc_g{ic}")
                daccr = dpool.tile([128, IC], bf16, tag="daccr",
                                   name=f"daccr{ic}")
                ops = [o_psum.tile([128, IC], f32, tag="op", name=f"op{ic}_{cb}")
                       for cb in range(2)]
                ptiles = {}
                sptiles = {}
                first_v = True
                first_g = True
                for g in range(NJB + 2):
                    if g < NJB:
                        # stage 0: S^T(g) on PE
                        sp = s_psum.tile([128, IC], f32, tag="sp",
                                         name=f"sp{ic}_{g}")
                        for h in range(2):
                            nc.tensor.matmul(
                                sp[:, h * 512:(h + 1) * 512],
                                lhsT=k_s[:, g * 128:(g + 1) * 128],
                                rhs=q_s[:, i0 + h * 512:i0 + (h + 1) * 512],
                                start=True, stop=True)
                        sptiles[g] = sp
                    if g == 0 and pending is not None:
                        emit_denom(pending)
                        emit_recips(pending)
                    if g == 1 and pending is not None:
                        emit_normalize(pending, 0)
                    if g == 2 and pending is not None:
                        emit_normalize(pending, 1)
                        pending = None
                    if 1 <= g <= NJB:
                        # stage 1: exp(g-1) on ScalarE; per-key bias t2
                        # rides in as the vT projection's extra column
                        je = g - 1
                        p = ppool.tile([128, IC], bf16, tag="p",
                                       name=f"p{ic}_{je}")
                        nc.scalar.activation(p, sptiles.pop(je), EXP,
                                             bias=vt_s[:, je, 256:257])
                        ptiles[je] = p
                    if g >= 2:
                        # stage 2: AV(g-2) on PE, denom add on DVE/GpSimd
                        jp = g - 2
                        p = ptiles[jp]
                        for cb in range(2):
                            for h in range(2):
                                nc.tensor.matmul(
                                    ops[cb][:, h * 512:(h + 1) * 512],
                                    lhsT=vt_s[:, jp, cb * 128:(cb + 1) * 128],
                                    rhs=p[:, h * 512:(h + 1) * 512],
                                    start=(jp == 0), stop=(jp == NJB - 1),
                                    skip_group_check=True)
                        if jp < NJB_ACC:
                            ptiles.pop(jp)
                            if jp not in GPS_JB:
                                if first_v:
                                    nc.vector.tensor_copy(dacc_v, p)
                                    first_v = False
                                else:
                                    nc.vector.tensor_add(dacc_v, dacc_v, p)
                            else:
                                if first_g:
                                    nc.gpsimd.tensor_copy(dacc_g, p)
                                    first_g = False
                                else:
                                    nc.gpsimd.tensor_add(dacc_g, dacc_g, p)
                            if jp == NJB_ACC - 1:
                                # both accumulators in: combine (bf16 out)
                                nc.vector.tensor_add(daccr, dacc_v, dacc_g)
                    if ic == NICHUNK - 1:
                        # final chunk: den head right after AV(30), its
                        # tail after AV(31), recips overlap the last AVs
                        if g == NJB:
                            pending = {
                                'ic': ic, 'daccr': daccr, 'ops': ops,
                                'ptail': [ptiles[j] for j in
                                          range(NJB_ACC, NJB)],
                            }
                            emit_denom_head(pending)
                        if g == NJB + 1:
                            emit_denom_tail(pending)
                            emit_recips(pending)
                if ic < NICHUNK - 1:
                    pending = {
                        'ic': ic, 'daccr': daccr, 'ops': ops,
                        'ptail': [ptiles[j] for j in range(NJB_ACC, NJB)],
                    }
            emit_normalize(pending, 0)
            emit_normalize(pending, 1)

    nc.compile()
    return nc


def _get_program():
    if "nc" not in _CACHE:
        _CACHE["nc"] = _build_program()
    return _CACHE["nc"]


def _host_prep(Wq, bq, Wk, bk, Wv, bv, gamma):
    g = float(np.asarray(gamma).reshape(-1)[0])
    Wq = np.asarray(Wq, np.float64)
    Wk = np.asarray(Wk, np.float64)
    bq = np.asarray(bq, np.float64)
    wqt = np.zeros((128, 2, 128), BF16)
    wkt = np.zeros((128, 2, 128), BF16)
    wqT = Wq.T.astype(np.float32).astype(BF16)  # (256, 32)
    wkT = Wk.T.astype(np.float32).astype(BF16)
    for kc in range(2):
        wqt[:, kc, :32] = wqT[kc * 128:(kc + 1) * 128]
        wkt[:, kc, :32] = wkT[kc * 128:(kc + 1) * 128]
    # v weights with gamma folded in, plus the t2 column (Wk^T bq) that
    # carries the per-key energy bias through the vT projection
    wvT = (g * np.asarray(Wv, np.float64)).T.astype(np.float32)  # (256c', 256c)
    t2col = (Wk.T @ bq).astype(np.float32)                       # (256,)
    wvt = np.zeros((128, 2, VW), BF16)
    for kc in range(2):
        sl = slice(kc * 128, (kc + 1) * 128)
        wvt[:, kc, 0:256] = wvT[sl].astype(BF16)
        wvt[:, kc, 256] = t2col[sl].astype(BF16)
    gbv_full = (g * np.asarray(bv, np.float64)).astype(np.float32)
    gbv = np.zeros((128, 2), np.float32)
    for cb in range(2):
        gbv[:, cb] = gbv_full[cb * 128:(cb + 1) * 128]

    wpk = np.zeros((128, WPACK_BYTES), np.uint8)
    wpk[:, 0:512] = wqt.reshape(128, 256).view(np.uint8)
    wpk[:, 512:1024] = wkt.reshape(128, 256).view(np.uint8)
    wpk[:, 1024:2056] = wvt.reshape(128, 2 * VW).view(np.uint8)
    wpk[:, 2056:2064] = gbv.view(np.uint8)
    return wpk


def _make_in_maps(pre_feat, post_feat, Wq, bq, Wk, bk, Wv, bv, gamma):
    pre_feat = np.asarray(pre_feat, np.float32)
    post_feat = np.asarray(post_feat, np.float32)
    pre_f = pre_feat.reshape(B, C, N)
    post_f = post_feat.reshape(B, C, N)
    wpk = _host_prep(Wq, bq, Wk, bk, Wv, bv, gamma)
    ones = np.ones((128, 128), BF16)
    in_maps = []
    for core in range(NCORES):
        b, half = core // 2, core % 2
        post_half = np.ascontiguousarray(
            post_f[b][:, half * NI:(half + 1) * NI])
        in_maps.append({
            "pre": np.ascontiguousarray(pre_f[b].astype(BF16)),
            "postr": post_half.astype(BF16),
            "post": post_half,
            "wpk": wpk,
            "ones": ones,
        })
    return in_maps


def kernel(pre_feat, post_feat, Wq, bq, Wk, bk, Wv, bv, gamma):
    from concourse.bass_utils import run_bass_kernel_spmd

    nc = _get_program()
    in_maps = _make_in_maps(pre_feat, post_feat, Wq, bq, Wk, bk, Wv, bv, gamma)
    res = run_bass_kernel_spmd(nc, in_maps, list(range(NCORES)))

    out_full = np.empty((B, C, N), np.float32)
    for core in range(NCORES):
        b, half = core // 2, core % 2
        out_full[b][:, half * NI:(half + 1) * NI] = res.results[core]["out"]
    return out_full.reshape(B, C, HH, WW)

